# revision 2
# baseline (speedup 1.0000x reference)
"""RGCN (mean-aggr) Trainium2 kernel, 8-core SPMD, dst-sharded. v3.

Baseline two-phase gather structure (HW-proven primitives only), plus:
  - bf16 datapath (x, staging, weights, means, output): half DMA, 4x matmul.
  - LPT-balanced dst->tile bins (max tile load ~96 <= 128): no 256-cap tiles.
  - Phase A (per sub x src-window) dma_gather writes B_d contiguously
    (p-major), phase B per-sweep dma_gather re-reads in tile-major order.
  - agg PSUM split A/B (3+4 banks) drained in parallel on DVE/Act; transform
    split accordingly; bias drain alternates engines; x^T preloaded to SBUF.
Output is out^T (bf16) per core in permuted dst order; host inverts.
"""

import heapq

import numpy as np
import ml_dtypes

BF16 = ml_dtypes.bfloat16

P = 128
N_NODES = 100000
N_EDGES = 600000
DIM = 128
NUM_RELS = 8
NCORES = 8

TILE_DST = 16
TILE_SLOTS = TILE_DST * NUM_RELS          # 128
NTILES = 784                              # per core
CW = NTILES * TILE_DST                    # 12544
NBINS = NCORES * NTILES                   # 6272
NSUB = 4                                  # dst subranges per core (phase A)
TPS = NTILES // NSUB                      # 196 tiles per sub
NQ = 4                                    # src windows
QW = 25088                                # src window width
SWEEP_TILES = 28
NSWEEPS = NTILES // SWEEP_TILES           # 28
SWEEP_SLOTS = SWEEP_TILES * TILE_SLOTS    # 3584
SWEEP_DST = SWEEP_TILES * TILE_DST        # 448
SPLIT_TILES = 12                          # aggA (3 PSUM banks)
SPLIT_SLOTS = SPLIT_TILES * TILE_SLOTS    # 1536
SPLIT_DST = SPLIT_TILES * TILE_DST        # 192
RESTB_SLOTS = SWEEP_SLOTS - SPLIT_SLOTS   # 2048 (4 banks)
SWEEPS_PER_SUB = TPS // SWEEP_TILES       # 7

_compiled = None


def _wrap16(idx_i16):
    n = len(idx_i16)
    w = idx_i16.reshape(n // 16, 16).T
    return np.ascontiguousarray(np.tile(w, (8, 1)))


def _build_program(CAPA, capt):
    import concourse.bacc as bacc
    import concourse.tile as tile
    from concourse import mybir

    ncols = (capt // P).astype(np.int64)
    col_off = np.concatenate([[0], np.cumsum(ncols)]).astype(np.int64)
    TOTCH = int(col_off[-1])
    TOTB = TOTCH * P
    AC = CAPA // P                        # chunk columns per (sub,q) bucket
    BROWS = NQ * CAPA + P                 # per-sub B rows (+zero row block)

    nc = bacc.Bacc(None, target_bir_lowering=False, debug=False)
    f32 = mybir.dt.float32
    bf16 = mybir.dt.bfloat16
    i16 = mybir.dt.int16
    i32 = mybir.dt.int32

    xg_d = nc.dram_tensor("xg", [NQ * QW, P], bf16, kind="ExternalInput")
    xT_d = nc.dram_tensor("xT", [P, CW], bf16, kind="ExternalInput")
    wcat_d = nc.dram_tensor("wcat", [P, NUM_RELS * P], bf16, kind="ExternalInput")
    wroot_d = nc.dram_tensor("wroot", [P, P], bf16, kind="ExternalInput")
    bias_d = nc.dram_tensor("bias", [P, 1], f32, kind="ExternalInput")
    gA_d = nc.dram_tensor("gA", [NSUB * NQ, P, CAPA // 16], i16, kind="ExternalInput")
    gB_d = nc.dram_tensor("gB", [P, TOTB // 16], i16, kind="ExternalInput")
    scol_d = nc.dram_tensor("scol", [P, TOTCH], f32, kind="ExternalInput")
    wgt_d = nc.dram_tensor("wgt", [P, TOTCH], f32, kind="ExternalInput")
    outT_d = nc.dram_tensor("outT", [P, CW], bf16, kind="ExternalOutput")

    B_d = [nc.dram_tensor(f"B{s}", [BROWS, P], bf16) for s in range(NSUB)]

    with tile.TileContext(nc) as tc:
        with (
            tc.tile_pool(name="const", bufs=1) as cpool,
            tc.tile_pool(name="stagA", bufs=3) as poolA,
            tc.tile_pool(name="stagB", bufs=4) as poolB,
            tc.tile_pool(name="spool", bufs=16) as spool,
            tc.tile_pool(name="mpool", bufs=4) as mpool,
            tc.tile_pool(name="opool", bufs=4) as opool,
            tc.tile_pool(name="ipool", bufs=4) as ipool,
            tc.tile_pool(name="psA", bufs=1, space="PSUM") as psA,
            tc.tile_pool(name="psO", bufs=1, space="PSUM") as psO,
        ):
            wcat = cpool.tile([P, NUM_RELS * P], bf16)
            wroot = cpool.tile([P, P], bf16)
            biast = cpool.tile([P, 1], f32)
            iota_i = cpool.tile([P, P], i32)
            iota_f = cpool.tile([P, P], bf16)
            zrow = cpool.tile([P, P], bf16)
            scolt = cpool.tile([P, TOTCH], f32)
            wgtt = cpool.tile([P, TOTCH], f32)
            xTt = cpool.tile([P, CW], bf16)

            nc.sync.dma_start(out=scolt[:], in_=scol_d[:])
            nc.sync.dma_start(out=wgtt[:], in_=wgt_d[:])
            nc.sync.dma_start(out=wcat[:], in_=wcat_d[:])
            nc.sync.dma_start(out=wroot[:], in_=wroot_d[:])
            nc.sync.dma_start(out=biast[:], in_=bias_d[:])
            nc.sync.dma_start(out=xTt[:], in_=xT_d[:])
            nc.gpsimd.iota(iota_i[:], pattern=[[1, P]], base=0, channel_multiplier=0)
            nc.vector.tensor_copy(out=iota_f[:], in_=iota_i[:])
            nc.vector.memset(zrow[:], 0.0)

            # ---- Phase A: src-window gathers -> B_s (contiguous p-major) ----
            for s in range(NSUB):
                nc.sync.dma_start(
                    out=B_d[s][NQ * CAPA:NQ * CAPA + P, :], in_=zrow[:])
                for q in range(NQ):
                    gA = ipool.tile([P, CAPA // 16], i16, tag="gA")
                    nc.sync.dma_start(out=gA[:], in_=gA_d[s * NQ + q])
                    stag = poolA.tile([P, AC, P], bf16, tag="stagA")
                    nc.gpsimd.dma_gather(
                        out_ap=stag[:],
                        in_ap=xg_d[QW * q:QW * (q + 1), :],
                        idxs_ap=gA[:],
                        num_idxs=CAPA, num_idxs_reg=CAPA, elem_size=P,
                        single_packet=False)
                    nc.sync.dma_start(
                        out=B_d[s][CAPA * q:CAPA * (q + 1), :].rearrange(
                            "(p a) d -> p a d", p=P),
                        in_=stag[:])

            # ---- Phase B: per-sweep gathers + segment + transform ----
            for s in range(NSWEEPS):
                sub = s // SWEEPS_PER_SUB
                t0 = s * SWEEP_TILES
                c0, c1 = int(col_off[t0]), int(col_off[t0 + SWEEP_TILES])
                swtok = (c1 - c0) * P
                gB = ipool.tile([P, swtok // 16], i16, tag="gB")
                nc.sync.dma_start(
                    out=gB[:], in_=gB_d[:, c0 * P // 16:c1 * P // 16])
                stag = poolB.tile([P, c1 - c0, P], bf16, tag="stagB")
                nc.gpsimd.dma_gather(
                    out_ap=stag[:], in_ap=B_d[sub][:, :], idxs_ap=gB[:],
                    num_idxs=swtok, num_idxs_reg=swtok, elem_size=P,
                    single_packet=False)

                aggA = psA.tile([P, SPLIT_SLOTS], f32, tag="aggA")
                aggB = psA.tile([P, RESTB_SLOTS], f32, tag="aggB")
                for tl in range(SWEEP_TILES):
                    t = t0 + tl
                    nch = int(ncols[t])
                    if tl < SPLIT_TILES:
                        aggv = aggA[:, tl * TILE_SLOTS:(tl + 1) * TILE_SLOTS]
                    else:
                        tb = tl - SPLIT_TILES
                        aggv = aggB[:, tb * TILE_SLOTS:(tb + 1) * TILE_SLOTS]
                    for j in range(nch):
                        col = int(col_off[t]) + j
                        Sc = spool.tile([P, P], bf16, tag="S")
                        nc.vector.tensor_scalar(
                            out=Sc[:], in0=iota_f[:],
                            scalar1=scolt[:, col:col + 1],
                            scalar2=wgtt[:, col:col + 1],
                            op0=mybir.AluOpType.is_equal,
                            op1=mybir.AluOpType.mult)
                        nc.tensor.matmul(
                            out=aggv,
                            lhsT=stag[:, col - c0, :], rhs=Sc[:],
                            start=(j == 0), stop=(j == nch - 1))

                meanA = mpool.tile([P, SPLIT_SLOTS], bf16, tag="meanA")
                meanB = mpool.tile([P, RESTB_SLOTS], bf16, tag="meanB")
                if s % 2 == 0:
                    nc.vector.tensor_copy(out=meanA[:], in_=aggA[:])
                else:
                    nc.scalar.activation(
                        out=meanA[:], in_=aggA[:],
                        func=mybir.ActivationFunctionType.Identity)
                nc.scalar.activation(
                    out=meanB[:], in_=aggB[:],
                    func=mybir.ActivationFunctionType.Identity)

                dst0 = s * SWEEP_DST
                outp = psO.tile([P, SWEEP_DST], f32)
                meanA_r = meanA[:].rearrange(
                    "p (dst rel) -> p dst rel", rel=NUM_RELS)
                meanB_r = meanB[:].rearrange(
                    "p (dst rel) -> p dst rel", rel=NUM_RELS)
                for r in range(NUM_RELS):
                    nc.tensor.matmul(
                        out=outp[:, :SPLIT_DST],
                        lhsT=wcat[:, r * P:(r + 1) * P],
                        rhs=meanA_r[:, :, r],
                        start=(r == 0), stop=False)
                nc.tensor.matmul(out=outp[:, :SPLIT_DST], lhsT=wroot[:],
                                 rhs=xTt[:, dst0:dst0 + SPLIT_DST],
                                 start=False, stop=True)
                for r in range(NUM_RELS):
                    nc.tensor.matmul(
                        out=outp[:, SPLIT_DST:],
                        lhsT=wcat[:, r * P:(r + 1) * P],
                        rhs=meanB_r[:, :, r],
                        start=(r == 0), stop=False)
                nc.tensor.matmul(out=outp[:, SPLIT_DST:], lhsT=wroot[:],
                                 rhs=xTt[:, dst0 + SPLIT_DST:dst0 + SWEEP_DST],
                                 start=False, stop=True)
                oT = opool.tile([P, SWEEP_DST], bf16, tag="oT")
                if s % 2 == 0:
                    nc.vector.tensor_scalar_add(
                        out=oT[:], in0=outp[:], scalar1=biast[:, 0:1])
                else:
                    nc.scalar.activation(
                        out=oT[:], in_=outp[:],
                        func=mybir.ActivationFunctionType.Identity,
                        bias=biast[:, 0:1])
                nc.sync.dma_start(out=outT_d[:, dst0:dst0 + SWEEP_DST], in_=oT[:])
    nc.compile()
    return nc


def _balance(cnt_dst):
    order = np.argsort(-cnt_dst, kind="stable")
    bin_of = np.empty(N_NODES, np.int64)
    pos_of = np.empty(N_NODES, np.int64)
    heap = [(0, b) for b in range(NBINS)]
    counts = np.zeros(NBINS, np.int32)
    for d in order:
        load, b = heapq.heappop(heap)
        bin_of[d] = b
        pos_of[d] = counts[b]
        counts[b] += 1
        if counts[b] < TILE_DST:
            heapq.heappush(heap, (load + int(cnt_dst[d]), b))
    return bin_of, pos_of


def _prepare(x, W, W_root, bias, edge_index, edge_type):
    src = np.asarray(edge_index[0], dtype=np.int64)
    dst = np.asarray(edge_index[1], dtype=np.int64)
    rel = np.asarray(edge_type, dtype=np.int64)

    cnt_slot = np.bincount(dst * NUM_RELS + rel, minlength=N_NODES * NUM_RELS)
    w_edge = (1.0 / np.maximum(cnt_slot[dst * NUM_RELS + rel], 1)).astype(np.float32)
    cnt_dst = np.bincount(dst, minlength=N_NODES).astype(np.int64)

    bin_of, pos_of = _balance(cnt_dst)
    bin_load = np.bincount(bin_of[dst], minlength=NBINS)
    brank = np.argsort(-bin_load, kind="stable")
    tile_of_bin = np.empty(NBINS, np.int64)
    core_of_bin = np.empty(NBINS, np.int64)
    tile_of_bin[brank] = np.arange(NBINS) // NCORES
    core_of_bin[brank] = np.arange(NBINS) % NCORES

    core_of_dst = core_of_bin[bin_of]
    tile_of_dst = tile_of_bin[bin_of]
    j_of_dst = pos_of

    e_core = core_of_dst[dst]
    e_tile = tile_of_dst[dst]
    e_scol = j_of_dst[dst] * NUM_RELS + rel
    e_sub = e_tile // TPS
    q = src // QW

    # phase A bucket caps (core, sub, q), shared across cores
    keyA = (e_core * NSUB + e_sub) * NQ + q
    bincA = np.bincount(keyA, minlength=NCORES * NSUB * NQ)
    CAPA = int(-(-bincA.max() // P) * P)
    CAPA = max(CAPA, P)
    AC = CAPA // P

    # per-tile chunk caps (shared across cores)
    keyT = e_core * NTILES + e_tile
    bincT = np.bincount(keyT, minlength=NCORES * NTILES).reshape(NCORES, NTILES)
    capt = np.maximum((-(-bincT.max(axis=0) // P)) * P, P).astype(np.int64)
    ncols = capt // P
    col_off = np.concatenate([[0], np.cumsum(ncols)]).astype(np.int64)
    TOTCH = int(col_off[-1])
    TOTB = TOTCH * P

    xg = np.zeros((NQ * QW, P), np.float32)
    xg[:N_NODES] = np.asarray(x, np.float32)
    xg = xg.astype(BF16)
    wcat = np.ascontiguousarray(
        np.asarray(W, np.float32).transpose(1, 0, 2).reshape(P, NUM_RELS * P)
    ).astype(BF16)
    wroot = np.ascontiguousarray(np.asarray(W_root, np.float32)).astype(BF16)
    biascol = np.asarray(bias, np.float32).reshape(P, 1)

    order_e = np.lexsort((e_scol, e_tile, e_core))
    in_maps = []
    dst_tables = []
    xnp = np.asarray(x, np.float32)
    for c in range(NCORES):
        sel = order_e[e_core[order_e] == c]
        csrc, cq, csub, cscol, ctile, cw = (
            src[sel], q[sel], e_sub[sel], e_scol[sel], e_tile[sel], w_edge[sel])

        # phase A: bucket by (sub, q); rank within bucket
        keyaq = csub * NQ + cq
        ordA = np.argsort(keyaq, kind="stable")
        gA = np.zeros((NSUB * NQ, P, CAPA // 16), np.int16)
        rankA = np.zeros(len(sel), np.int64)
        for sq in range(NSUB * NQ):
            members = ordA[keyaq[ordA] == sq]
            n = len(members)
            assert n <= CAPA, (n, CAPA)
            rankA[members] = np.arange(n)
            idx = np.zeros(CAPA, np.int16)
            idx[:n] = (csrc[members] - QW * cq[members]).astype(np.int16)
            gA[sq] = _wrap16(idx)
        # B row for each edge: stag row r=(p,a) written p-major
        brow = CAPA * cq + (rankA % P) * AC + rankA // P

        # phase B tokens: tile-major with per-tile caps
        gB_lin = np.full(TOTB, NQ * CAPA, np.int64)  # default: zero row
        gB_lin += np.arange(TOTB) % P
        scol_lin = np.full(TOTB, -1.0, np.float32)
        wgt_lin = np.zeros(TOTB, np.float32)
        ordT = np.argsort(ctile, kind="stable")
        tcounts = np.bincount(ctile, minlength=NTILES)
        tstart = np.concatenate([[0], np.cumsum(tcounts)])[:-1]
        arangepos = np.empty(len(sel), np.int64)
        arangepos[ordT] = np.arange(len(sel))
        rank_in_tile = arangepos - tstart[ctile]
        assert (rank_in_tile < capt[ctile]).all()
        tok = col_off[ctile] * P + rank_in_tile
        gB_lin[tok] = brow
        scol_lin[tok] = cscol.astype(np.float32)
        wgt_lin[tok] = cw
        assert gB_lin.max() < 32768

        mask = core_of_dst == c
        dst_ids = np.nonzero(mask)[0]
        cols = tile_of_dst[dst_ids] * TILE_DST + j_of_dst[dst_ids]
        dst_table = np.full(CW, -1, np.int64)
        dst_table[cols] = dst_ids
        valid = dst_table >= 0
        xT = np.zeros((P, CW), np.float32)
        xT[:, valid] = xnp[dst_table[valid]].T
        xT = xT.astype(BF16)

        # scol/wgt as [P, TOTCH]: token (p, col) -> p = tok % P, col = tok // P
        scol_arr = np.ascontiguousarray(
            scol_lin.reshape(TOTCH, P).T)
        wgt_arr = np.ascontiguousarray(
            wgt_lin.reshape(TOTCH, P).T)

        in_maps.append({
            "xg": xg,
            "xT": xT,
            "wcat": wcat,
            "wroot": wroot,
            "bias": biascol,
            "gA": gA,
            "gB": _wrap16(gB_lin.astype(np.int16)),
            "scol": scol_arr,
            "wgt": wgt_arr,
        })
        dst_tables.append(dst_table)
    return in_maps, dst_tables, CAPA, capt


LAST_EXEC_NS = None


def kernel(x, W, W_root, bias, edge_index, edge_type):
    global _compiled, LAST_EXEC_NS
    import os
    from concourse.bass_utils import run_bass_kernel_spmd

    in_maps, dst_tables, CAPA, capt = _prepare(
        x, W, W_root, bias, edge_index, edge_type)
    key = (CAPA, capt.tobytes())
    if _compiled is None or _compiled[0] != key:
        nc = _build_program(CAPA, capt)
        _compiled = (key, nc)
    nc = _compiled[1]

    trace = bool(int(os.environ.get("BASS_PROFILE", "0")))
    r = run_bass_kernel_spmd(nc, in_maps, list(range(NCORES)), trace=trace)
    if trace and getattr(r, "exec_time_ns", None) is not None:
        LAST_EXEC_NS = r.exec_time_ns
    res = r.results
    out = np.empty((N_NODES, DIM), np.float32)
    for c in range(NCORES):
        outT = np.asarray(res[c]["outT"]).astype(np.float32)
        dt = dst_tables[c]
        valid = dt >= 0
        out[dt[valid]] = outT[:, valid].T
    return out
